# revision 33
# baseline (speedup 1.0000x reference)
"""Balanced dice loss (histogram binning) on 8 Trainium2 NeuronCores.

Math: with t ∈ {0,1} and p = sigmoid(x), the loss needs four global sums:
    S_t   = Σ t            (the bincount)
    S_pt  = Σ p·t
    S_pp  = Σ p²
    S_ppt = Σ p²·t
Then with c1 = S_t, c0 = N − c1, w0 = 1/(c0+s)², w1 = 1/(c1+s)²:
    intersection = w1·S_pt
    denominator  = w0·(S_pp − S_ppt) + w1·(S_ppt + c1)
    dice = 1 − (2·I + s)/(D + s)

Device kernel (data-parallel over 8 cores, batch-sharded), per [128,F] tile:
    ACT : p = sigmoid(x) (bf16); tb = copy(t) int32→bf16 with row-accum
          → S_t (one pass does the dtype conversion AND the bincount)
    DVE : u = p·tb, w = u·p (= p²·t), sq = p·p — all-bf16 2× perf mode
    PE  : ones[128,128] @ 512-col chunks of u, w, sq → three PSUM
          column-sum accumulation chains (S_pt, S_ppt, S_pp), each
          alternating two banks to pipeline the RMW
p/tb and u/w/sq live in two merged pool tiles (fewer pool allocations →
shorter end-of-kernel semaphore teardown). The last tile computes
u = p·t from the int32 tile directly (1×) so u doesn't wait on the S_t
copy, and runs its DVE work in 512-col sub-slices so the PE chains
drain as chunks are produced. Partials are DMA'd out; host reduces in
float64.
"""

import numpy as np

import concourse.bacc as bacc
import concourse.mybir as mybir
from concourse.bass_utils import run_bass_kernel_spmd
from concourse.tile import TileContext

N_CORES = 8
P = 128
TOTAL = 32 * 1024 * 1024  # elements in the full problem
PER_CORE = TOTAL // N_CORES  # 4,194,304
FREE = PER_CORE // P  # 32,768 f32 per partition
F = 2048  # tile free-dim
NT = FREE // F  # 16 tiles per core
MMN = 512  # matmul moving free-dim (one PSUM bank; ISA max)
NCH = F // MMN  # matmul chunks per tile
SMOOTH = 1e-05

_nc_cache = None


def _build_bass():
    nc = bacc.Bacc(None, target_bir_lowering=False)
    x = nc.dram_tensor("input", [P, FREE], mybir.dt.float32, kind="ExternalInput")
    t = nc.dram_tensor("target", [P, FREE], mybir.dt.int32, kind="ExternalInput")
    o_sums = nc.dram_tensor(
        "o_sums", [1, 6 * MMN], mybir.dt.float32, kind="ExternalOutput"
    )
    o_st = nc.dram_tensor("o_st", [P, NT], mybir.dt.float32, kind="ExternalOutput")

    with TileContext(nc) as tc:
        with (
            tc.tile_pool(name="work", bufs=2) as pool,
            tc.tile_pool(name="stats", bufs=1) as spool,
            tc.tile_pool(name="ps", bufs=1, space="PSUM") as psum,
        ):
            s_t = spool.tile([P, NT], mybir.dt.float32)
            ones = spool.tile([P, P], mybir.dt.bfloat16, tag="ones")
            ps_pt_a = psum.tile([P, MMN], mybir.dt.float32, tag="ps_pt_a")
            ps_pt_b = psum.tile([P, MMN], mybir.dt.float32, tag="ps_pt_b")
            ps_ppt_a = psum.tile([P, MMN], mybir.dt.float32, tag="ps_ppt_a")
            ps_ppt_b = psum.tile([P, MMN], mybir.dt.float32, tag="ps_ppt_b")
            ps_pp_a = psum.tile([P, MMN], mybir.dt.float32, tag="ps_pp_a")
            ps_pp_b = psum.tile([P, MMN], mybir.dt.float32, tag="ps_pp_b")

            # emit the first tile's loads before the ones-memset so the
            # sync queue reaches them as early as possible
            xts, tts = [], []
            for i in range(NT):
                xt = pool.tile([P, F], mybir.dt.float32, tag="xt", bufs=6)
                tt = pool.tile([P, F], mybir.dt.int32, tag="tt", bufs=6)
                nc.sync.dma_start(xt[:], x[:, i * F : (i + 1) * F])
                nc.sync.dma_start(tt[:], t[:, i * F : (i + 1) * F])
                xts.append(xt)
                tts.append(tt)
                if i == 0:
                    nc.any.memset(ones, 1.0)

            for i in range(NT):
                xt, tt = xts[i], tts[i]
                ptb = pool.tile([P, 2 * F], mybir.dt.bfloat16, tag="ptb", bufs=3)
                uwsq = pool.tile([P, 3 * F], mybir.dt.bfloat16, tag="uwsq")
                p_, tb = ptb[:, :F], ptb[:, F:]
                u, w, sq = uwsq[:, :F], uwsq[:, F : 2 * F], uwsq[:, 2 * F :]

                # p = sigmoid(x); tb = float(t) with S_t row-accum   [ACT]
                nc.scalar.activation(
                    p_, xt[:], mybir.ActivationFunctionType.Sigmoid
                )
                nc.scalar.activation(
                    tb,
                    tt[:],
                    mybir.ActivationFunctionType.Copy,
                    accum_out=s_t[:, i : i + 1],
                )

                chains = (
                    (0, (ps_pt_a, ps_pt_b)),  # u chunks
                    (F, (ps_ppt_a, ps_ppt_b)),  # w chunks
                    (2 * F, (ps_pp_a, ps_pp_b)),  # sq chunks
                )

                def mms(j0, nch):
                    # column-sum accumulation chains; each chain
                    # alternates two PSUM banks to pipeline the RMW  [PE]
                    for base, banks in chains:
                        for jj in range(nch):
                            j = j0 + jj
                            nc.tensor.matmul(
                                banks[j % 2][:],
                                ones[:],
                                uwsq[:, base + j * MMN : base + (j + 1) * MMN],
                                start=(i == 0 and j < 2),
                                stop=(i == NT - 1 and j >= NCH - 2),
                            )

                if i < NT - 1:
                    # u = p·t, w = u·p = p²t, sq = p² (bf16 2x mode) [DVE]
                    nc.vector.tensor_tensor(
                        out=u, in0=p_, in1=tb, op=mybir.AluOpType.mult
                    )
                    nc.vector.tensor_tensor(
                        out=w, in0=u, in1=p_, op=mybir.AluOpType.mult
                    )
                    nc.vector.tensor_tensor(
                        out=sq, in0=p_, in1=p_, op=mybir.AluOpType.mult
                    )
                    mms(0, NCH)
                else:
                    # last tile: u = p·t from the int32 tile (1x) so u
                    # doesn't wait on the S_t copy, and 512-col sub-slices
                    # so the PE chains drain as chunks are produced
                    for c in range(NCH):
                        sl = slice(c * MMN, (c + 1) * MMN)
                        nc.vector.tensor_tensor(
                            out=u[:, sl], in0=p_[:, sl], in1=tt[:, sl],
                            op=mybir.AluOpType.mult,
                        )
                        nc.vector.tensor_tensor(
                            out=w[:, sl], in0=u[:, sl], in1=p_[:, sl],
                            op=mybir.AluOpType.mult,
                        )
                        nc.vector.tensor_tensor(
                            out=sq[:, sl], in0=p_[:, sl], in1=p_[:, sl],
                            op=mybir.AluOpType.mult,
                        )
                        mms(c, 1)

            fin = spool.tile([1, 6 * MMN], mybir.dt.float32, tag="fin")
            for k, ps in enumerate(
                (ps_pt_a, ps_pt_b, ps_ppt_a, ps_ppt_b, ps_pp_a, ps_pp_b)
            ):
                dst = fin[:, k * MMN : (k + 1) * MMN]
                if k % 2 == 0:
                    nc.vector.tensor_copy(dst, ps[0:1, :])
                else:
                    nc.scalar.copy(dst, ps[0:1, :])
                if k == 3:
                    # fire the S_pt/S_ppt half while the S_pp copies run so
                    # the final (teardown-gating) DMA is only the small rest
                    nc.sync.dma_start(o_sums[:, : 4 * MMN], fin[:, : 4 * MMN])
            nc.sync.dma_start(o_sums[:, 4 * MMN :], fin[:, 4 * MMN :])
            nc.sync.dma_start(o_st[:], s_t[:])
    nc.finalize()
    return nc


def _get_nc():
    global _nc_cache
    if _nc_cache is None:
        _nc_cache = _build_bass()
    return _nc_cache


def kernel(input, target, _trace=False):
    x = np.ascontiguousarray(np.asarray(input, dtype=np.float32)).reshape(
        N_CORES, P, FREE
    )
    t = np.ascontiguousarray(np.asarray(target, dtype=np.int32)).reshape(
        N_CORES, P, FREE
    )
    in_maps = [{"input": x[i], "target": t[i]} for i in range(N_CORES)]

    nc = _get_nc()
    res = run_bass_kernel_spmd(
        nc, in_maps, core_ids=list(range(N_CORES)), trace=_trace
    )
    kernel.last_results = res

    s_pt = s_ppt = s_pp = s_t = 0.0
    for r in res.results:
        sums = r["o_sums"].astype(np.float64)
        s_pt += float(sums[0, 0 : 2 * MMN].sum())
        s_ppt += float(sums[0, 2 * MMN : 4 * MMN].sum())
        s_pp += float(sums[0, 4 * MMN :].sum())
        s_t += float(r["o_st"].astype(np.float64).sum())

    c1 = float(s_t)
    c0 = float(TOTAL - s_t)
    w0 = 1.0 / (c0 + SMOOTH) ** 2
    w1 = 1.0 / (c1 + SMOOTH) ** 2
    intersection = w1 * s_pt
    denominator = w0 * (s_pp - s_ppt) + w1 * (s_ppt + c1)
    dice = 1.0 - (2.0 * intersection + SMOOTH) / (denominator + SMOOTH)
    return np.asarray(dice, dtype=np.float32)
